# revision 19
# baseline (speedup 1.0000x reference)
"""AdaptiveAttention on 8 TRN2 NeuronCores.

Model (b=2, n=2048, dim=1024, 16 heads x 64, NUM_W=4 adaptive weights):
    gates = softmax(x @ Wg)                                  # [b, n, 4]
    qkv_w = x @ Wqkv  (packed (qkv, h, d, w))                # per w: q,k,v
    q,k,v = sum_w gates_w * qkv_w                            # gated combine
    out   = softmax(q k^T / sqrt(64)) v                      # per head
    y     = sum_w gates_w * (out @ Wout_w)                   # gated out-proj

Sharding: core c -> batch beta=c//4, head-group g=c%4 (4 heads each).
Each core computes a partial out-projection over its 256 dim_inner rows;
the host sums the 4 partials per batch (partial-sum output sharding).

Key kernel ideas:
 - Single transpose of x -> xT [dim, tok]; all matmuls then run natively.
 - Host pre-permutes the weight column packings so every PE operand is
   contiguous in SBUF: Wqkv (q h d w)->(q w h d) for the stationary
   slices, Wout (d w)->(eh w e) so the out-proj MOVING operand streams
   contiguously (strided moving ops run at half PE rate), Wg tiled to
   [128, c w] so its load is a single fast DMA.
 - Gates computed transposed on the PE: logits gT [w, tok] accumulate
   over dim chunks with wg stationary (no per-chunk LDWEIGHTS), exp on
   ACT, then per-token sums + per-w broadcasts via tiny-K matmuls.
 - Gated combine folded into the QKV matmul: accumulate over (dim-chunk,
   w) of Wqkv_w^T @ (xT * G_w) in PSUM - the w-combine is free on
   TensorE. Per-block interleave of gates+QKV so the PE starts at ~5us.
 - v returned to natural [keys, (h d|1)] layout with SBUF->SBUF DMA
   transposes (frees the PE), ones column appended for softmax row-sums.
 - Scores computed transposed, ST = kT^T qT -> [keys, q], ROW-TILED:
   the two heads of a pair run as concurrent K=64 matmuls in the top and
   bottom halves of the PE array (tile_position), halving ST cost.
   exp(ST) tiles are directly the lhsT for the PV matmul.
 - Stage C's out-projection matmuls are interleaved into the attention
   kc loops as PE filler; PSUM drains ride on DVE, softmax exp on ACT,
   so all three engines stay busy. bf16 everywhere on the PE, f32 PSUM.
"""

import os

import numpy as np

P = 128
N_TOK = 2048          # tokens per batch
DIM = 1024
DH = 64               # head dim
W = 4                 # adaptive weights
NH_LOC = 4            # heads per core
FEAT = NH_LOC * DH    # 256 local dim_inner feats
CB = DIM // P         # 8 dim chunks
TB = 4                # token blocks (512 each)
TBS = N_TOK // TB     # 512
KC = N_TOK // P       # 16 key chunks
QB = 4                # query blocks (512 each)
QBS = N_TOK // QB     # 512
VW = DH + 1           # 65: v columns + ones column for row-sums

_BUILT = None


def _split_waits(nc, keep=1):
    """Workaround: this neuronxcc walrus build rejects instructions carrying
    more than one sync wait ("Too many sync wait commands") on several codegen
    paths (Drain, CollectiveCompute, PSEUDO_DMA_DIRECT2D, ...). Hoist excess
    waits onto dedicated single-wait EventSemaphore carriers inserted
    immediately before the instruction on the same engine."""
    import concourse.mybir as mybir

    for fn in nc.m.functions:
        for bb in fn.blocks:
            new_list = []
            for inst in bb.instructions:
                si = inst.sync_info
                waits = list(si.on_wait) if si is not None else []
                if len(waits) > keep:
                    extra, kept = waits[keep:], waits[:keep]
                    for j, w in enumerate(extra):
                        c = mybir.InstEventSemaphore(
                            name=f"{inst.name}-pw{j}", ins=[], outs=[],
                            sync_info=mybir.SyncInfo(on_wait=[w], on_update=[]),
                        )
                        c.engine = inst.engine
                        new_list.append(c)
                    si.on_wait.clear()
                    for w in kept:
                        si.on_wait.append(w)
                new_list.append(inst)
            bb.instructions[:] = new_list


def _patch_tile_exit():
    """Trim the TileContext exit: split the drain's waits (walrus single-wait
    limit) and drop the final all-engine barrier - after the first barrier no
    engine has further instructions, so only the semaphore clears remain."""
    import concourse.tile as tile
    from concourse.vector_clock import ScopedClock
    if getattr(tile.TileContext, "_exit_trimmed", False):
        return

    def _drain_and_barrier(self, tick_clock, wait_clock):
        nc = self.nc
        probe = nc.sync.nop()
        wait_clock.add_sem_waits(probe.ins, ScopedClock({None: tick_clock.global_clock}))
        si = probe.ins.sync_info
        waits = list(si.on_wait) if si is not None else []
        if si is not None:
            si.on_wait.clear()
        handles = {h.name: h for h in self.sems.allocated().values()}
        for w in waits:
            h = handles.get(w.ant_name)
            assert h is not None, f"no semaphore handle named {w.ant_name}"
            nc.sync.wait_ge(h, w.wait_value)
        nc.sync.drain()
        nc.all_engine_barrier()
        assert self.sems is not None
        popped = nc._tile_sem_poison_stack.pop()
        assert popped is self._sem_poison
        nc.clear_and_free_semaphores(list(self.sems.allocated().values()))

    tile.TileContext._drain_and_barrier = _drain_and_barrier
    tile.TileContext._exit_trimmed = True


def _build():
    import concourse.bass as bass
    import concourse.mybir as mybir
    import concourse.tile as tile
    from concourse.masks import make_identity

    F32 = mybir.dt.float32
    BF16 = mybir.dt.bfloat16
    EXP = mybir.ActivationFunctionType.Exp
    MUL = mybir.AluOpType.mult

    _patch_tile_exit()
    nc = bass.Bass()
    x_ext = nc.declare_dram_parameter("x", [N_TOK, DIM], BF16, isOutput=False)
    wqkv_ext = nc.declare_dram_parameter("wqkv", [DIM, 3 * FEAT * W], BF16, isOutput=False)
    wg_ext = nc.declare_dram_parameter("wg", [P, CB * W], BF16, isOutput=False)
    wout_ext = nc.declare_dram_parameter("wout", [FEAT, DIM * W], BF16, isOutput=False)
    out_ext = nc.declare_dram_parameter("out", [N_TOK, DIM], BF16, isOutput=True)

    with tile.TileContext(nc) as tc:
        with (
            tc.tile_pool(name="const", bufs=1) as constp,
            tc.tile_pool(name="big", bufs=1) as bigp,
        ):
            ident = constp.tile([P, P], F32, tag="idf", name="idf")
            make_identity(nc, ident[:])
            ident_bf = constp.tile([P, P], BF16, tag="idb", name="idb")
            nc.vector.tensor_copy(ident_bf[:], ident[:])
            # bf16 selector (row 0 ones) for the row-sum broadcasts
            sel0b = constp.tile([P, P], BF16, tag="sel0b", name="sel0b")
            nc.gpsimd.memset(sel0b[:], 0.0)
            nc.gpsimd.affine_select(
                out=sel0b[:], in_=sel0b[:],
                compare_op=mybir.AluOpType.not_equal, fill=1.0,
                base=0, pattern=[[0, P]], channel_multiplier=1)
            # 4-partition selectors (row w ones) + all-ones for the gates
            # sum/broadcast matmuls
            sels4 = []
            for w in range(W):
                s = constp.tile([W, P], BF16, tag=f"s4{w}", name=f"s4{w}")
                nc.gpsimd.memset(s[:], 0.0)
                nc.gpsimd.affine_select(
                    out=s[:], in_=s[:],
                    compare_op=mybir.AluOpType.not_equal, fill=1.0,
                    base=-w, pattern=[[0, P]], channel_multiplier=1)
                sels4.append(s)
            ones4 = constp.tile([W, P], BF16, tag="ones4", name="ones4")
            nc.vector.memset(ones4[:], 1.0)

            # resident stage outputs
            G = [bigp.tile([P, N_TOK], BF16, tag=f"G{w}", name=f"G{w}") for w in range(W)]
            qT = [bigp.tile([P, N_TOK], BF16, tag=f"qT{i}", name=f"qT{i}") for i in range(2)]
            kT = [bigp.tile([P, N_TOK], BF16, tag=f"kT{i}", name=f"kT{i}") for i in range(2)]
            outT = [bigp.tile([P, N_TOK], BF16, tag=f"oT{i}", name=f"oT{i}") for i in range(2)]
            # v in natural layout: [keys_in_chunk, (kc h (d|1))]
            v_all = bigp.tile([P, KC * NH_LOC * VW], BF16, tag="vall", name="vall")
            va = v_all.rearrange("p (kc h v) -> p kc h v", kc=KC, h=NH_LOC, v=VW)
            nc.vector.memset(va[:, :, :, DH:VW], 1.0)

            # ================= Stage A: xT, gates, QKV =================
            with (
                tc.tile_pool(name="wq", bufs=1) as wqp,
                tc.tile_pool(name="xtp", bufs=1) as xtp,
                tc.tile_pool(name="scrA", bufs=3) as scrp,
                tc.tile_pool(name="yw", bufs=3) as ywp,
                tc.tile_pool(name="ps_qkv", bufs=1, space="PSUM") as ps_qkv,
                tc.tile_pool(name="ps_tr", bufs=2, space="PSUM") as ps_tr,
            ):
                # x transposed per token-block: ONE DMA transpose per block
                # yields all 8 dim-chunks as xblk[t][p, c, tok] (3D-out
                # enumeration is block-major - probed). Few big DMAs matter:
                # the tile framework's DMA-sem reuse serializes many small
                # DMAs across queues. All transposes stay on the SP queue
                # (concurrent transposes on two HWDGE queues corrupt each
                # other - shared XBAR state).
                xblk = [xtp.tile([P, CB * TBS], BF16, tag=f"xb{t}", name=f"xb{t}")
                        for t in range(TB)]
                xv = [xblk[t].rearrange("p (c n) -> p c n", c=CB, n=TBS)
                      for t in range(TB)]
                for t in range(TB):
                    nc.sync.dma_start(
                        xv[t][:, :, :], x_ext[t * TBS:(t + 1) * TBS, :],
                        transpose=True)
                # wg + wqkv + wout ride the GpSimd SW-DGE queue: big DMAs on
                # the SP/ACT HWDGE queues stall those engines' sequencers on
                # queue slots. wqkv layout per chunk: [128, (qkv)(w)(h)(d)]
                wg_sb = constp.tile([P, CB * W], BF16, tag="wg", name="wg")
                nc.gpsimd.dma_start(wg_sb[:], wg_ext[:])
                wqkv_sb = [wqp.tile([P, 3 * FEAT * W], BF16, tag=f"wqkv{c}", name=f"wqkv{c}")
                           for c in range(CB)]
                for c in range(CB):
                    nc.gpsimd.dma_start(wqkv_sb[c][:], wqkv_ext[c * P:(c + 1) * P, :])

                def gates_chain(t):
                    # thunks computing G[*] for block t: logits gEt [w, tok]
                    # accumulated over dim chunks (wg chunk stationary), exp
                    # on ACT, per-token sums + per-w broadcasts via tiny-K
                    # matmuls, reciprocal as exp(-ln(x)) on ACT (the 3.4us
                    # DVE reciprocal would sit on the QKV critical path)
                    ts = t * TBS
                    box = {}
                    thunks = []

                    def alloc():
                        box["gEt"] = ps_tr.tile([W, TBS], F32, tag="tr", name="gEt")
                    thunks.append(alloc)
                    for c in range(CB):
                        def mm(c=c):
                            nc.tensor.matmul(
                                box["gEt"][:], wg_sb[:, c * W:(c + 1) * W],
                                xv[t][:, c, :],
                                start=(c == 0), stop=(c == CB - 1))
                        thunks.append(mm)

                    def expf():
                        gE = scrp.tile([W, TBS], BF16, tag="gE", name="gE")
                        nc.scalar.activation(gE[:], box["gEt"][:], EXP)
                        box["gE"] = gE
                        box["sumb"] = ps_tr.tile([P, TBS], F32, tag="tr", name="sumb")
                        nc.tensor.matmul(
                            box["sumb"][:], ones4[:], gE[:], start=True, stop=True)
                    thunks.append(expf)

                    def recipf():
                        rbl = scrp.tile([P, TBS], F32, tag="rbl", name="rbl")
                        nc.scalar.activation(
                            rbl[:], box["sumb"][:], mybir.ActivationFunctionType.Ln)
                        rbs = scrp.tile([P, TBS], F32, tag="rbs", name="rbs")
                        nc.scalar.activation(rbs[:], rbl[:], EXP, scale=-1.0)
                        box["rbs"] = rbs
                    thunks.append(recipf)
                    for w in range(W):
                        def ebf(w=w):
                            eb = ps_tr.tile([P, TBS], F32, tag="tr", name="eb")
                            nc.tensor.matmul(
                                eb[:], sels4[w][:], box["gE"][:],
                                start=True, stop=True)
                            nc.vector.tensor_tensor(
                                G[w][:, ts: ts + TBS], eb[:], box["rbs"][:], MUL)
                        thunks.append(ebf)
                    return thunks

                gpend = []

                def gpump(n):
                    for _ in range(min(n, len(gpend))):
                        gpend.pop(0)()

                for th in gates_chain(0):
                    th()
                for t in range(TB):
                    ts = t * TBS
                    if t + 1 < TB:
                        # software-pipeline the next block's gates into this
                        # block's QKV matmul stream
                        gpend.extend(gates_chain(t + 1))
                    # QKV: accumulate over (c, w) of Wqkv_w^T @ (xT_c * G_w)
                    pq = [ps_qkv.tile([P, TBS], F32, tag=f"pq{i}", name=f"pq{i}") for i in range(2)]
                    pk = [ps_qkv.tile([P, TBS], F32, tag=f"pk{i}", name=f"pk{i}") for i in range(2)]
                    pv = [ps_qkv.tile([P, TBS], F32, tag=f"pv{i}", name=f"pv{i}") for i in range(2)]
                    for c in range(CB):
                        for w in range(W):
                            ci = c * W + w
                            yw = ywp.tile([P, TBS], BF16, tag="yw", name="yw")
                            nc.vector.tensor_tensor(
                                yw[:], xv[t][:, c, :], G[w][:, ts: ts + TBS], MUL)
                            wv = wqkv_sb[c].rearrange(
                                "p (q w h d) -> p q w h d", q=3, w=W, h=NH_LOC, d=DH)
                            st = (c == 0 and w == 0)
                            sp = (c == CB - 1 and w == W - 1)
                            for hp in range(2):
                                nc.tensor.matmul(
                                    pq[hp][:], wv[:, 0, w, 2 * hp:2 * hp + 2, :], yw[:],
                                    start=st, stop=sp)
                                nc.tensor.matmul(
                                    pk[hp][:], wv[:, 1, w, 2 * hp:2 * hp + 2, :], yw[:],
                                    start=st, stop=sp)
                                nc.tensor.matmul(
                                    pv[hp][:], wv[:, 2, w, 2 * hp:2 * hp + 2, :], yw[:],
                                    start=st, stop=sp)
                            # pump the next block's gates only in the second
                            # half of this block, so the pumped matmuls never
                            # head-of-line-block the PE on a late xT arrival
                            if ci >= 14:
                                gpump(2)
                    vT_sb = [scrp.tile([P, TBS], BF16, tag=f"vT{i}", name=f"vT{i}") for i in range(2)]
                    for hp in range(2):
                        nc.scalar.copy(qT[hp][:, ts: ts + TBS], pq[hp][:])
                        nc.scalar.copy(kT[hp][:, ts: ts + TBS], pk[hp][:])
                        nc.scalar.copy(vT_sb[hp][:], pv[hp][:])
                    # v back to natural layout [keys, (h, d)]: PE transposes
                    # (the DMA-transpose ucode scrambles offset SBUF sources)
                    for tt in range(4):
                        kc = t * 4 + tt
                        for hp in range(2):
                            vtp = ps_tr.tile([P, TBS], BF16, tag="tr", name="vtp")
                            nc.tensor.transpose(
                                vtp[:, 0:P], vT_sb[hp][:, tt * P:(tt + 1) * P],
                                ident_bf[:])
                            nc.scalar.copy(
                                va[:, kc, 2 * hp:2 * hp + 2, 0:DH],
                                vtp[:, 0:P].rearrange("p (h d) -> p h d", h=2, d=DH))

            # ========= Stage B+C: attention fused with out-projection =====
            # Stage C's matmuls for query-block qb-1 are issued between the
            # attention blocks (software pipelining) so the PE never waits on
            # the normalize/gating elementwise chain.
            with (
                tc.tile_pool(name="pt", bufs=2) as ptp,
                tc.tile_pool(name="scrB", bufs=3) as scrbp,
                tc.tile_pool(name="woutp", bufs=1) as woutp,
                tc.tile_pool(name="owp", bufs=2) as owp,
                tc.tile_pool(name="zp", bufs=2) as zp,
                tc.tile_pool(name="ps_st", bufs=2, space="PSUM") as ps_st,
                tc.tile_pool(name="ps_pv", bufs=2, space="PSUM") as ps_pv,
                tc.tile_pool(name="ps_z", bufs=2, space="PSUM") as ps_z,
            ):
                wout_sb = [woutp.tile([P, DIM * W], BF16, tag=f"wo{fc}", name=f"wo{fc}")
                           for fc in range(2)]
                for fc in range(2):
                    nc.gpsimd.dma_start(
                        wout_sb[fc][:], wout_ext[fc * P:(fc + 1) * P, :])
                rs = [scrbp.tile([P, QBS], BF16, tag=f"rs{r}", name=f"rs{r}",
                                 bufs=1)
                      for r in range(16)]
                for r in range(16):
                    nc.vector.memset(rs[r][:], 0.0)
                ow = {}

                def finalize_half(qb, oi):
                    # normalize outT[oi][:, qb] by the softmax row-sums and
                    # apply the output gates for that half (heads 2oi, 2oi+1)
                    qs = qb * QBS
                    rb = ps_st.tile([P, 2 * QBS], F32, tag="st", name="st")
                    nc.tensor.matmul(
                        rb[0:DH, 0:QBS], sel0b[:, 0:DH],
                        rs[qb * 4 + 2 * oi][:], start=True, stop=True)
                    nc.tensor.matmul(
                        rb[DH:P, 0:QBS], sel0b[:, 0:DH],
                        rs[qb * 4 + 2 * oi + 1][:], start=True, stop=True,
                        tile_position=(0, 64))
                    # reciprocal as exp(-ln(x)) on ACT: ~0.9us vs 3.3us for
                    # the DVE reciprocal (row-sums are positive, ~1e-6 rel err)
                    rbc = scrbp.tile([P, QBS], F32, tag="rbc", name="rbc",
                                     bufs=2)
                    nc.scalar.activation(
                        rbc[:], rb[:, 0:QBS], mybir.ActivationFunctionType.Ln)
                    rbs = scrbp.tile([P, QBS], F32, tag="rbs", name="rbs")
                    nc.scalar.activation(rbs[:], rbc[:], EXP, scale=-1.0)
                    sl = outT[oi][:, qs: qs + QBS]
                    nc.vector.tensor_tensor(sl, sl, rbs[:], MUL)
                    for w in range(W):
                        o = owp.tile([P, QBS], BF16, tag=f"ow{oi}{w}",
                                     name=f"ow{oi}{w}")
                        nc.vector.tensor_tensor(
                            o[:], outT[oi][:, qs: qs + QBS],
                            G[w][:, qs: qs + QBS], MUL)
                        ow[(qb, oi, w)] = o

                pending = []

                def zproj_thunks(qb):
                    # out-projection for qb as a flat list of issue thunks so
                    # its matmuls can be interleaved into the attention kc
                    # loops as TensorE filler work
                    thunks = []
                    for tt in range(4):
                        box = []

                        def alloc(box=box):
                            box.append([ps_z.tile([P, 512], F32, tag="z",
                                                  name="z")
                                        for _ in range(2)])
                        thunks.append(alloc)
                        for fc in range(2):
                            for w in range(W):
                                for half in range(2):
                                    def mm(box=box, tt=tt, fc=fc, w=w,
                                           half=half, qb=qb):
                                        wv = wout_sb[fc].rearrange(
                                            "p (eh w e) -> p eh w e",
                                            eh=2, w=W, e=512)
                                        nc.tensor.matmul(
                                            box[0][half][:],
                                            ow[(qb, fc, w)][:, tt * P:(tt + 1) * P],
                                            wv[:, half, w, :],
                                            start=(fc == 0 and w == 0),
                                            stop=(fc == 1 and w == W - 1))
                                    thunks.append(mm)

                        def fin(box=box, tt=tt, qb=qb):
                            zps = box.pop()
                            ttk = qb * 4 + tt
                            zs = zp.tile([P, DIM], BF16, tag="zs", name="zs")
                            for half in range(2):
                                nc.vector.tensor_copy(
                                    zs[:, half * 512:(half + 1) * 512],
                                    zps[half][:])
                            nc.sync.dma_start(
                                out_ext[ttk * P:(ttk + 1) * P, :], zs[:])
                        thunks.append(fin)
                    return thunks

                def pump(n):
                    for _ in range(min(n, len(pending))):
                        pending.pop(0)()

                # Head-pair phases, software-pipelined one pair deep: the PV
                # matmuls of pair i-1 are interleaved into pair i's ST loop so
                # the PE stays dense while ACT computes the exps.
                pairs = [(qb, hp) for qb in range(QB) for hp in range(2)]
                prev = None  # (qb, hp, pts, po_tiles)

                def pv_finalize(qb_p, hp_p, pts_p, pos_p):
                    for hh in range(2):
                        h = hp_p * 2 + hh
                        qs_p = qb_p * QBS
                        oi, orow = divmod(h * DH, P)
                        nc.vector.tensor_copy(
                            outT[oi][orow: orow + DH, qs_p: qs_p + QBS],
                            pos_p[hh][0:DH, :])
                        nc.vector.tensor_copy(
                            rs[qb_p * 4 + h][0:1, :], pos_p[hh][DH:VW, :])

                for i, (qb, hp) in enumerate(pairs):
                    qs = qb * QBS
                    pts = ptp.tile([P, KC * 2 * QBS], BF16, tag="pt", name="pt")
                    if prev is not None:
                        qb_p, hp_p, pts_p, _ = prev
                        pos_p = [ps_pv.tile([VW, QBS], F32, tag="po", name="po")
                                 for _ in range(2)]
                        prev = (qb_p, hp_p, pts_p, pos_p)
                    for kc in range(KC):
                        ks = kc * P
                        # scores for the two heads of the pair as concurrent
                        # K=64 row-tiles in the top/bottom array halves
                        s2 = ps_st.tile([P, 2 * QBS], F32, tag="st", name="st")
                        nc.tensor.matmul(
                            s2[:, 0:QBS], kT[hp][0:DH, ks: ks + P],
                            qT[hp][0:DH, qs: qs + QBS],
                            start=True, stop=True, tile_position=(0, 0))
                        nc.tensor.matmul(
                            s2[:, QBS:2 * QBS], kT[hp][DH:P, ks: ks + P],
                            qT[hp][DH:P, qs: qs + QBS],
                            start=True, stop=True, tile_position=(64, 0))
                        if prev is not None:
                            qb_p, hp_p, pts_p, pos_p = prev
                            for hh in range(2):
                                h = hp_p * 2 + hh
                                nc.tensor.matmul(
                                    pos_p[hh][:],
                                    va[:, kc, h, :],
                                    pts_p[:, kc * 2 * QBS + hh * QBS:
                                          kc * 2 * QBS + (hh + 1) * QBS],
                                    start=(kc == 0), stop=(kc == KC - 1))
                        nc.scalar.activation(
                            pts[:, kc * 2 * QBS:(kc + 1) * 2 * QBS], s2[:],
                            EXP, scale=0.125)
                        pump(3)
                    if prev is not None:
                        qb_p, hp_p, pts_p, pos_p = prev
                        pv_finalize(qb_p, hp_p, pts_p, pos_p)
                        if hp_p == 1 and qb_p < QB - 1:
                            # deferred into the next pair's kc loop via the
                            # pump so the rb matmuls never bubble the PE queue
                            # at the pair boundary (zproj for this qb is
                            # enqueued a pair later - FIFO keeps the order)
                            pending.append(lambda qb=qb_p: finalize_half(qb, 0))
                            pending.append(lambda qb=qb_p: finalize_half(qb, 1))
                        if hp_p == 0 and qb_p > 0:
                            # out-projection for the qb finalized one pair ago:
                            # its gated ow tiles have had a full phase to land
                            pending.extend(zproj_thunks(qb_p - 1))
                        if hp_p == 0 and qb_p == QB - 1:
                            # last qb: heads 0-1 can normalize a phase early,
                            # shortening the tail's elementwise chain
                            finalize_half(QB - 1, 0)
                    prev = (qb, hp, pts, None)
                # drain the last pair
                qb_p, hp_p, pts_p, _ = prev
                pos_p = [ps_pv.tile([VW, QBS], F32, tag="po", name="po")
                         for _ in range(2)]
                for kc in range(KC):
                    for hh in range(2):
                        h = hp_p * 2 + hh
                        nc.tensor.matmul(
                            pos_p[hh][:], va[:, kc, h, :],
                            pts_p[:, kc * 2 * QBS + hh * QBS:
                                  kc * 2 * QBS + (hh + 1) * QBS],
                            start=(kc == 0), stop=(kc == KC - 1))
                    pump(2)
                pv_finalize(qb_p, hp_p, pts_p, pos_p)
                finalize_half(qb_p, 1)
                pump(len(pending))
                for th in zproj_thunks(qb_p):
                    th()

    _split_waits(nc)
    return nc


def _get_built():
    global _BUILT
    if _BUILT is None:
        _BUILT = _build()
    return _BUILT


def kernel(x, Wqkv, Wg, Wout, mask=None, **_ignored):
    """Full inputs in, full output out. mask is all-ones by construction and
    is ignored (attention over an all-true mask is mask-free)."""
    from concourse.bass_utils import run_bass_kernel_spmd

    import ml_dtypes
    bf16 = ml_dtypes.bfloat16
    x = np.asarray(x, dtype=np.float32).astype(bf16)
    Wqkv = np.asarray(Wqkv, dtype=np.float32).astype(bf16)
    Wg = np.asarray(Wg, dtype=np.float32).astype(bf16)
    Wout = np.asarray(Wout, dtype=np.float32).astype(bf16)
    b = x.shape[0]

    # Wg [dim, w] -> [128, (chunk, w)] partition-tiled for a fast plain DMA
    wg_host = np.ascontiguousarray(
        Wg.reshape(CB, P, W).transpose(1, 0, 2).reshape(P, CB * W))

    in_maps = []
    for c in range(8):
        beta, g = c // 4, c % 4
        cols = []
        for q in range(3):
            blk = Wqkv[:, (q * 16 + 4 * g) * 256:(q * 16 + 4 * g + 4) * 256]
            # local column packing (h, d, w) -> (w, h, d): stationary matmul
            # slices become contiguous 128-column runs
            blk = blk.reshape(DIM, NH_LOC, DH, W).transpose(0, 3, 1, 2)
            cols.append(blk.reshape(DIM, FEAT * W))
        wo = Wout[g * 256:(g + 1) * 256, :]
        # (eh, e, w) -> (eh, w, e): moving operand streams contiguously
        wo = wo.reshape(FEAT, 2, 512, W).transpose(0, 1, 3, 2)
        in_maps.append({
            "x": np.ascontiguousarray(x[beta]),
            "wqkv": np.ascontiguousarray(np.concatenate(cols, axis=1)),
            "wg": wg_host,
            "wout": np.ascontiguousarray(wo.reshape(FEAT, DIM * W)),
        })

    nc = _get_built()
    trace = bool(int(os.environ.get("KBENCH_TRACE", "0")))
    res = run_bass_kernel_spmd(nc, in_maps, core_ids=list(range(8)), trace=trace)
    kernel.last_exec_time_ns = res.exec_time_ns

    out = np.zeros((b, N_TOK, DIM), dtype=np.float32)
    for c in range(8):
        out[c // 4] += res.results[c]["out"].astype(np.float32)
    return out


# revision 23
# speedup vs baseline: 1.1105x; 1.1105x over previous
"""AdaptiveAttention on 8 TRN2 NeuronCores.

Model (b=2, n=2048, dim=1024, 16 heads x 64, NUM_W=4 adaptive weights):
    gates = softmax(x @ Wg)                                  # [b, n, 4]
    qkv_w = x @ Wqkv  (packed (qkv, h, d, w))                # per w: q,k,v
    q,k,v = sum_w gates_w * qkv_w                            # gated combine
    out   = softmax(q k^T / sqrt(64)) v                      # per head
    y     = sum_w gates_w * (out @ Wout_w)                   # gated out-proj

Sharding: core c -> batch beta=c//4, head-group g=c%4 (4 heads each).
Each core computes a partial out-projection over its 256 dim_inner rows;
the host sums the 4 partials per batch (partial-sum output sharding).

Key kernel ideas:
 - Single transpose of x -> xT [dim, tok]; all matmuls then run natively.
 - Host pre-permutes the weight column packings so every PE operand is
   contiguous in SBUF: Wqkv (q h d w)->(q w h d) for the stationary
   slices, Wout (d w)->(eh w e) so the out-proj MOVING operand streams
   contiguously (strided moving ops run at half PE rate), Wg tiled to
   [128, c w] so its load is a single fast DMA.
 - Gates computed transposed on the PE: logits gT [w, tok] accumulate
   over dim chunks with wg stationary (no per-chunk LDWEIGHTS), exp on
   ACT, then per-token sums + per-w broadcasts via tiny-K matmuls.
 - Gated combine folded into the QKV matmul: accumulate over (dim-chunk,
   w) of Wqkv_w^T @ (xT * G_w) in PSUM - the w-combine is free on
   TensorE. Per-block interleave of gates+QKV so the PE starts at ~5us.
 - v returned to natural [keys, (h d|1)] layout with SBUF->SBUF DMA
   transposes (frees the PE), ones column appended for softmax row-sums.
 - Scores computed transposed, ST = kT^T qT -> [keys, q], ROW-TILED:
   the two heads of a pair run as concurrent K=64 matmuls in the top and
   bottom halves of the PE array (tile_position), halving ST cost.
   exp(ST) tiles are directly the lhsT for the PV matmul.
 - Stage C's out-projection matmuls are interleaved into the attention
   kc loops as PE filler; PSUM drains ride on DVE, softmax exp on ACT,
   so all three engines stay busy. bf16 everywhere on the PE, f32 PSUM.
"""

import os

import numpy as np

P = 128
N_TOK = 2048          # tokens per batch
DIM = 1024
DH = 64               # head dim
W = 4                 # adaptive weights
NH_LOC = 4            # heads per core
FEAT = NH_LOC * DH    # 256 local dim_inner feats
CB = DIM // P         # 8 dim chunks
TB = 4                # token blocks (512 each)
TBS = N_TOK // TB     # 512
KC = N_TOK // P       # 16 key chunks
QB = 4                # query blocks (512 each)
QBS = N_TOK // QB     # 512
VW = DH + 1           # 65: v columns + ones column for row-sums

_BUILT = None


def _split_waits(nc, keep=1):
    """Workaround: this neuronxcc walrus build rejects instructions carrying
    more than one sync wait ("Too many sync wait commands") on several codegen
    paths (Drain, CollectiveCompute, PSEUDO_DMA_DIRECT2D, ...). Hoist excess
    waits onto dedicated single-wait EventSemaphore carriers inserted
    immediately before the instruction on the same engine."""
    import concourse.mybir as mybir

    for fn in nc.m.functions:
        for bb in fn.blocks:
            new_list = []
            for inst in bb.instructions:
                si = inst.sync_info
                waits = list(si.on_wait) if si is not None else []
                if len(waits) > keep:
                    extra, kept = waits[keep:], waits[:keep]
                    for j, w in enumerate(extra):
                        c = mybir.InstEventSemaphore(
                            name=f"{inst.name}-pw{j}", ins=[], outs=[],
                            sync_info=mybir.SyncInfo(on_wait=[w], on_update=[]),
                        )
                        c.engine = inst.engine
                        new_list.append(c)
                    si.on_wait.clear()
                    for w in kept:
                        si.on_wait.append(w)
                new_list.append(inst)
            bb.instructions[:] = new_list


def _patch_tile_exit():
    """Trim the TileContext exit: split the drain's waits (walrus single-wait
    limit) and drop the final all-engine barrier - after the first barrier no
    engine has further instructions, so only the semaphore clears remain."""
    import concourse.tile as tile
    from concourse.vector_clock import ScopedClock
    if getattr(tile.TileContext, "_exit_trimmed", False):
        return

    def _drain_and_barrier(self, tick_clock, wait_clock):
        nc = self.nc
        probe = nc.sync.nop()
        wait_clock.add_sem_waits(probe.ins, ScopedClock({None: tick_clock.global_clock}))
        si = probe.ins.sync_info
        waits = list(si.on_wait) if si is not None else []
        if si is not None:
            si.on_wait.clear()
        handles = {h.name: h for h in self.sems.allocated().values()}
        for w in waits:
            h = handles.get(w.ant_name)
            assert h is not None, f"no semaphore handle named {w.ant_name}"
            nc.sync.wait_ge(h, w.wait_value)
        nc.sync.drain()
        nc.all_engine_barrier()
        assert self.sems is not None
        popped = nc._tile_sem_poison_stack.pop()
        assert popped is self._sem_poison
        nc.clear_and_free_semaphores(list(self.sems.allocated().values()))

    tile.TileContext._drain_and_barrier = _drain_and_barrier
    tile.TileContext._exit_trimmed = True


def _build():
    import concourse.bass as bass
    import concourse.mybir as mybir
    import concourse.tile as tile
    from concourse.masks import make_identity

    F32 = mybir.dt.float32
    BF16 = mybir.dt.bfloat16
    EXP = mybir.ActivationFunctionType.Exp
    MUL = mybir.AluOpType.mult

    _patch_tile_exit()
    nc = bass.Bass()
    x_ext = nc.declare_dram_parameter("x", [N_TOK, DIM], BF16, isOutput=False)
    wqkv_ext = nc.declare_dram_parameter("wqkv", [DIM, 3 * FEAT * W], BF16, isOutput=False)
    wg_ext = nc.declare_dram_parameter("wg", [P, CB * W], BF16, isOutput=False)
    wout_ext = nc.declare_dram_parameter("wout", [FEAT, DIM * W], BF16, isOutput=False)
    out_ext = nc.declare_dram_parameter("out", [N_TOK, DIM], BF16, isOutput=True)

    with tile.TileContext(nc) as tc:
        with (
            tc.tile_pool(name="const", bufs=1) as constp,
            tc.tile_pool(name="big", bufs=1) as bigp,
        ):
            ident = constp.tile([P, P], F32, tag="idf", name="idf")
            make_identity(nc, ident[:])
            ident_bf = constp.tile([P, P], BF16, tag="idb", name="idb")
            nc.vector.tensor_copy(ident_bf[:], ident[:])

            # resident stage outputs
            G = [bigp.tile([P, N_TOK], BF16, tag=f"G{w}", name=f"G{w}") for w in range(W)]
            qT = [bigp.tile([P, N_TOK], BF16, tag=f"qT{i}", name=f"qT{i}") for i in range(2)]
            kT = [bigp.tile([P, N_TOK], BF16, tag=f"kT{i}", name=f"kT{i}") for i in range(2)]
            outT = [bigp.tile([P, N_TOK], BF16, tag=f"oT{i}", name=f"oT{i}") for i in range(2)]
            # v in natural layout: [keys_in_chunk, (kc h (d|1))]
            v_all = bigp.tile([P, KC * NH_LOC * VW], BF16, tag="vall", name="vall")
            va = v_all.rearrange("p (kc h v) -> p kc h v", kc=KC, h=NH_LOC, v=VW)
            nc.vector.memset(va[:, :, :, DH:VW], 1.0)

            # ================= Stage A: xT, gates, QKV =================
            with (
                tc.tile_pool(name="wq", bufs=1) as wqp,
                tc.tile_pool(name="xtp", bufs=1) as xtp,
                tc.tile_pool(name="scrA", bufs=3) as scrp,
                tc.tile_pool(name="yw", bufs=3) as ywp,
                tc.tile_pool(name="ps_qkv", bufs=1, space="PSUM") as ps_qkv,
                tc.tile_pool(name="ps_tr", bufs=2, space="PSUM") as ps_tr,
            ):
                # x transposed per token-block: ONE DMA transpose per block
                # yields all 8 dim-chunks as xblk[t][p, c, tok] (3D-out
                # enumeration is block-major - probed). Few big DMAs matter:
                # the tile framework's DMA-sem reuse serializes many small
                # DMAs across queues. All transposes stay on the SP queue
                # (concurrent transposes on two HWDGE queues corrupt each
                # other - shared XBAR state).
                xblk = [xtp.tile([P, CB * TBS], BF16, tag=f"xb{t}", name=f"xb{t}")
                        for t in range(TB)]
                xv = [xblk[t].rearrange("p (c n) -> p c n", c=CB, n=TBS)
                      for t in range(TB)]
                # block 0 arrives as plain row-loads and is transposed on the
                # (startup-idle) PE below - ~6us faster than the DMA-transpose
                # whose per-instruction issue cost is high. Blocks 1-3 use one
                # DMA transpose each (3D out, block-major - probed), all on
                # the SP queue (concurrent transposes on two HWDGE queues
                # corrupt each other - shared XBAR state).
                xnat = [xtp.tile([P, DIM], BF16, tag=f"xn{k}", name=f"xn{k}")
                        for k in range(4)]
                for k in range(4):
                    nc.sync.dma_start(xnat[k][:], x_ext[k * P:(k + 1) * P, :])
                for t in range(1, TB):
                    nc.sync.dma_start(
                        xv[t][:, :, :], x_ext[t * TBS:(t + 1) * TBS, :],
                        transpose=True)
                # wg + wqkv + wout ride the GpSimd SW-DGE queue: big DMAs on
                # the SP/ACT HWDGE queues stall those engines' sequencers on
                # queue slots. wqkv layout per chunk: [128, (qkv)(w)(h)(d)]
                wg_sb = constp.tile([P, CB * W], BF16, tag="wg", name="wg")
                nc.gpsimd.dma_start(wg_sb[:], wg_ext[:])
                wqkv_sb = [wqp.tile([P, 3 * FEAT * W], BF16, tag=f"wqkv{c}", name=f"wqkv{c}")
                           for c in range(CB)]
                for c in range(CB):
                    nc.gpsimd.dma_start(wqkv_sb[c][:], wqkv_ext[c * P:(c + 1) * P, :])
                # selector constants AFTER the dma issues (same gpsimd stream)
                # so the weight transfers start as early as possible
                sel0b = constp.tile([P, P], BF16, tag="sel0b", name="sel0b")
                nc.gpsimd.memset(sel0b[:], 0.0)
                nc.gpsimd.affine_select(
                    out=sel0b[:], in_=sel0b[:],
                    compare_op=mybir.AluOpType.not_equal, fill=1.0,
                    base=0, pattern=[[0, P]], channel_multiplier=1)
                sels4 = []
                for w in range(W):
                    s = constp.tile([W, P], BF16, tag=f"s4{w}", name=f"s4{w}")
                    nc.gpsimd.memset(s[:], 0.0)
                    nc.gpsimd.affine_select(
                        out=s[:], in_=s[:],
                        compare_op=mybir.AluOpType.not_equal, fill=1.0,
                        base=-w, pattern=[[0, P]], channel_multiplier=1)
                    sels4.append(s)
                ones4 = constp.tile([W, P], BF16, tag="ones4", name="ones4")
                nc.vector.memset(ones4[:], 1.0)

                def gates_chain(t):
                    # thunks computing G[*] for block t: logits gEt [w, tok]
                    # accumulated over dim chunks (wg chunk stationary), exp
                    # on ACT, per-token sums + per-w broadcasts via tiny-K
                    # matmuls, reciprocal as exp(-ln(x)) on ACT (the 3.4us
                    # DVE reciprocal would sit on the QKV critical path)
                    ts = t * TBS
                    box = {}
                    thunks = []

                    def alloc():
                        box["gEt"] = ps_tr.tile([W, TBS], F32, tag="tr", name="gEt")
                    thunks.append(alloc)
                    for c in range(CB):
                        def mm(c=c):
                            nc.tensor.matmul(
                                box["gEt"][:], wg_sb[:, c * W:(c + 1) * W],
                                xv[t][:, c, :],
                                start=(c == 0), stop=(c == CB - 1))
                        thunks.append(mm)

                    def expf():
                        gE = scrp.tile([W, TBS], BF16, tag="gE", name="gE")
                        nc.scalar.activation(gE[:], box["gEt"][:], EXP)
                        box["gE"] = gE
                        box["sumb"] = ps_tr.tile([P, TBS], F32, tag="tr", name="sumb")
                        nc.tensor.matmul(
                            box["sumb"][:], ones4[:], gE[:], start=True, stop=True)
                    thunks.append(expf)

                    def recipf():
                        rbl = scrp.tile([P, TBS], F32, tag="rbl", name="rbl")
                        nc.scalar.activation(
                            rbl[:], box["sumb"][:], mybir.ActivationFunctionType.Ln)
                        rbs = scrp.tile([P, TBS], F32, tag="rbs", name="rbs")
                        nc.scalar.activation(rbs[:], rbl[:], EXP, scale=-1.0)
                        box["rbs"] = rbs
                    thunks.append(recipf)
                    for w in range(W):
                        def ebf(w=w):
                            eb = ps_tr.tile([P, TBS], F32, tag="tr", name="eb")
                            nc.tensor.matmul(
                                eb[:], sels4[w][:], box["gE"][:],
                                start=True, stop=True)
                            nc.vector.tensor_tensor(
                                G[w][:, ts: ts + TBS], eb[:], box["rbs"][:], MUL)
                        thunks.append(ebf)
                    return thunks

                gpend = []

                def gpump(n):
                    for _ in range(min(n, len(gpend))):
                        gpend.pop(0)()

                # bootstrap: PE-transpose block 0 from the natural-row tiles,
                # interleaved with block 0's gates logit matmuls per chunk
                g0 = gates_chain(0)
                g0[0]()
                for c in range(CB):
                    for k in range(4):
                        tps = ps_tr.tile([P, TBS], BF16, tag="tr", name="xtps")
                        nc.tensor.transpose(
                            tps[:, 0:P], xnat[k][:, c * P:(c + 1) * P],
                            ident_bf[:])
                        if k % 2 == 0:
                            nc.scalar.copy(
                                xv[0][:, c, k * P:(k + 1) * P], tps[:, 0:P])
                        else:
                            nc.vector.tensor_copy(
                                xv[0][:, c, k * P:(k + 1) * P], tps[:, 0:P])
                    g0[1 + c]()
                for th in g0[1 + CB:]:
                    th()
                for t in range(TB):
                    ts = t * TBS
                    if t + 1 < TB:
                        # software-pipeline the next block's gates into this
                        # block's QKV matmul stream
                        gpend.extend(gates_chain(t + 1))
                    # QKV: accumulate over (c, w) of Wqkv_w^T @ (xT_c * G_w)
                    pq = [ps_qkv.tile([P, TBS], F32, tag=f"pq{i}", name=f"pq{i}") for i in range(2)]
                    pk = [ps_qkv.tile([P, TBS], F32, tag=f"pk{i}", name=f"pk{i}") for i in range(2)]
                    pv = [ps_qkv.tile([P, TBS], F32, tag=f"pv{i}", name=f"pv{i}") for i in range(2)]
                    for c in range(CB):
                        for w in range(W):
                            ci = c * W + w
                            yw = ywp.tile([P, TBS], BF16, tag="yw", name="yw")
                            nc.vector.tensor_tensor(
                                yw[:], xv[t][:, c, :], G[w][:, ts: ts + TBS], MUL)
                            wv = wqkv_sb[c].rearrange(
                                "p (q w h d) -> p q w h d", q=3, w=W, h=NH_LOC, d=DH)
                            st = (c == 0 and w == 0)
                            sp = (c == CB - 1 and w == W - 1)
                            for hp in range(2):
                                nc.tensor.matmul(
                                    pq[hp][:], wv[:, 0, w, 2 * hp:2 * hp + 2, :], yw[:],
                                    start=st, stop=sp)
                                nc.tensor.matmul(
                                    pk[hp][:], wv[:, 1, w, 2 * hp:2 * hp + 2, :], yw[:],
                                    start=st, stop=sp)
                                nc.tensor.matmul(
                                    pv[hp][:], wv[:, 2, w, 2 * hp:2 * hp + 2, :], yw[:],
                                    start=st, stop=sp)
                            # pump the next block's gates only in the second
                            # half of this block, so the pumped matmuls never
                            # head-of-line-block the PE on a late xT arrival
                            if ci >= 14:
                                gpump(2)
                    vT_sb = [scrp.tile([P, TBS], BF16, tag=f"vT{i}", name=f"vT{i}") for i in range(2)]
                    for hp in range(2):
                        nc.scalar.copy(qT[hp][:, ts: ts + TBS], pq[hp][:])
                        nc.scalar.copy(kT[hp][:, ts: ts + TBS], pk[hp][:])
                        nc.scalar.copy(vT_sb[hp][:], pv[hp][:])
                    # v back to natural layout [keys, (h, d)]: PE transposes
                    # (the DMA-transpose ucode scrambles offset SBUF sources)
                    for tt in range(4):
                        kc = t * 4 + tt
                        for hp in range(2):
                            vtp = ps_tr.tile([P, TBS], BF16, tag="tr", name="vtp")
                            nc.tensor.transpose(
                                vtp[:, 0:P], vT_sb[hp][:, tt * P:(tt + 1) * P],
                                ident_bf[:])
                            nc.scalar.copy(
                                va[:, kc, 2 * hp:2 * hp + 2, 0:DH],
                                vtp[:, 0:P].rearrange("p (h d) -> p h d", h=2, d=DH))

            # ========= Stage B+C: attention fused with out-projection =====
            # Stage C's matmuls for query-block qb-1 are issued between the
            # attention blocks (software pipelining) so the PE never waits on
            # the normalize/gating elementwise chain.
            with (
                tc.tile_pool(name="pt", bufs=2) as ptp,
                tc.tile_pool(name="scrB", bufs=3) as scrbp,
                tc.tile_pool(name="woutp", bufs=1) as woutp,
                tc.tile_pool(name="owp", bufs=2) as owp,
                tc.tile_pool(name="zp", bufs=2) as zp,
                tc.tile_pool(name="ps_st", bufs=2, space="PSUM") as ps_st,
                tc.tile_pool(name="ps_pv", bufs=2, space="PSUM") as ps_pv,
                tc.tile_pool(name="ps_z", bufs=2, space="PSUM") as ps_z,
            ):
                wout_sb = [woutp.tile([P, DIM * W], BF16, tag=f"wo{fc}", name=f"wo{fc}")
                           for fc in range(2)]
                for fc in range(2):
                    nc.gpsimd.dma_start(
                        wout_sb[fc][:], wout_ext[fc * P:(fc + 1) * P, :])
                rs = [scrbp.tile([P, QBS], BF16, tag=f"rs{r}", name=f"rs{r}",
                                 bufs=1)
                      for r in range(16)]
                for r in range(16):
                    nc.vector.memset(rs[r][:], 0.0)
                ow = {}

                def finalize_half(qb, oi):
                    # normalize outT[oi][:, qb] by the softmax row-sums and
                    # apply the output gates for that half (heads 2oi, 2oi+1)
                    qs = qb * QBS
                    rb = ps_st.tile([P, 2 * QBS], F32, tag="st", name="st")
                    nc.tensor.matmul(
                        rb[0:DH, 0:QBS], sel0b[:, 0:DH],
                        rs[qb * 4 + 2 * oi][:], start=True, stop=True)
                    nc.tensor.matmul(
                        rb[DH:P, 0:QBS], sel0b[:, 0:DH],
                        rs[qb * 4 + 2 * oi + 1][:], start=True, stop=True,
                        tile_position=(0, 64))
                    # reciprocal as exp(-ln(x)) on ACT: ~0.9us vs 3.3us for
                    # the DVE reciprocal (row-sums are positive, ~1e-6 rel err)
                    rbc = scrbp.tile([P, QBS], F32, tag="rbc", name="rbc",
                                     bufs=2)
                    nc.scalar.activation(
                        rbc[:], rb[:, 0:QBS], mybir.ActivationFunctionType.Ln)
                    rbs = scrbp.tile([P, QBS], F32, tag="rbs", name="rbs")
                    nc.scalar.activation(rbs[:], rbc[:], EXP, scale=-1.0)
                    sl = outT[oi][:, qs: qs + QBS]
                    nc.vector.tensor_tensor(sl, sl, rbs[:], MUL)
                    for w in range(W):
                        o = owp.tile([P, QBS], BF16, tag=f"ow{oi}{w}",
                                     name=f"ow{oi}{w}")
                        nc.vector.tensor_tensor(
                            o[:], outT[oi][:, qs: qs + QBS],
                            G[w][:, qs: qs + QBS], MUL)
                        ow[(qb, oi, w)] = o

                pending = []

                def zproj_thunks(qb):
                    # out-projection for qb as a flat list of issue thunks so
                    # its matmuls can be interleaved into the attention kc
                    # loops as TensorE filler work
                    thunks = []
                    for tt in range(4):
                        box = []

                        def alloc(box=box):
                            box.append([ps_z.tile([P, 512], F32, tag="z",
                                                  name="z")
                                        for _ in range(2)])
                        thunks.append(alloc)
                        for fc in range(2):
                            for w in range(W):
                                for half in range(2):
                                    def mm(box=box, tt=tt, fc=fc, w=w,
                                           half=half, qb=qb):
                                        wv = wout_sb[fc].rearrange(
                                            "p (eh w e) -> p eh w e",
                                            eh=2, w=W, e=512)
                                        nc.tensor.matmul(
                                            box[0][half][:],
                                            ow[(qb, fc, w)][:, tt * P:(tt + 1) * P],
                                            wv[:, half, w, :],
                                            start=(fc == 0 and w == 0),
                                            stop=(fc == 1 and w == W - 1))
                                    thunks.append(mm)

                        def fin(box=box, tt=tt, qb=qb):
                            zps = box.pop()
                            ttk = qb * 4 + tt
                            zs = zp.tile([P, DIM], BF16, tag="zs", name="zs")
                            for half in range(2):
                                nc.vector.tensor_copy(
                                    zs[:, half * 512:(half + 1) * 512],
                                    zps[half][:])
                            nc.sync.dma_start(
                                out_ext[ttk * P:(ttk + 1) * P, :], zs[:])
                        thunks.append(fin)
                    return thunks

                def pump(n):
                    for _ in range(min(n, len(pending))):
                        pending.pop(0)()

                # Head-pair phases, software-pipelined one pair deep: the PV
                # matmuls of pair i-1 are interleaved into pair i's ST loop so
                # the PE stays dense while ACT computes the exps.
                pairs = [(qb, hp) for qb in range(QB) for hp in range(2)]
                prev = None  # (qb, hp, pts, po_tiles)

                def pv_finalize(qb_p, hp_p, pts_p, pos_p):
                    for hh in range(2):
                        h = hp_p * 2 + hh
                        qs_p = qb_p * QBS
                        oi, orow = divmod(h * DH, P)
                        nc.vector.tensor_copy(
                            outT[oi][orow: orow + DH, qs_p: qs_p + QBS],
                            pos_p[hh][0:DH, :])
                        nc.vector.tensor_copy(
                            rs[qb_p * 4 + h][0:1, :], pos_p[hh][DH:VW, :])

                for i, (qb, hp) in enumerate(pairs):
                    qs = qb * QBS
                    pts = ptp.tile([P, KC * 2 * QBS], BF16, tag="pt", name="pt")
                    if prev is not None:
                        qb_p, hp_p, pts_p, _ = prev
                        pos_p = [ps_pv.tile([VW, QBS], F32, tag="po", name="po")
                                 for _ in range(2)]
                        prev = (qb_p, hp_p, pts_p, pos_p)
                    for kc in range(KC):
                        ks = kc * P
                        # scores for the two heads of the pair as concurrent
                        # K=64 row-tiles in the top/bottom array halves
                        s2 = ps_st.tile([P, 2 * QBS], F32, tag="st", name="st")
                        nc.tensor.matmul(
                            s2[:, 0:QBS], kT[hp][0:DH, ks: ks + P],
                            qT[hp][0:DH, qs: qs + QBS],
                            start=True, stop=True, tile_position=(0, 0))
                        nc.tensor.matmul(
                            s2[:, QBS:2 * QBS], kT[hp][DH:P, ks: ks + P],
                            qT[hp][DH:P, qs: qs + QBS],
                            start=True, stop=True, tile_position=(64, 0))
                        if prev is not None:
                            qb_p, hp_p, pts_p, pos_p = prev
                            for hh in range(2):
                                h = hp_p * 2 + hh
                                nc.tensor.matmul(
                                    pos_p[hh][:],
                                    va[:, kc, h, :],
                                    pts_p[:, kc * 2 * QBS + hh * QBS:
                                          kc * 2 * QBS + (hh + 1) * QBS],
                                    start=(kc == 0), stop=(kc == KC - 1))
                        nc.scalar.activation(
                            pts[:, kc * 2 * QBS:(kc + 1) * 2 * QBS], s2[:],
                            EXP, scale=0.125)
                        pump(3)
                    if prev is not None:
                        qb_p, hp_p, pts_p, pos_p = prev
                        pv_finalize(qb_p, hp_p, pts_p, pos_p)
                        if hp_p == 1 and qb_p < QB - 1:
                            # eager on purpose: deferring these into the pump
                            # makes their Ln/Exp compete with the saturated
                            # EXP stream mid-pair (measured 70us worse)
                            finalize_half(qb_p, 0)
                            finalize_half(qb_p, 1)
                        if hp_p == 0 and qb_p > 0:
                            # out-projection for the qb finalized one pair ago:
                            # its gated ow tiles have had a full phase to land
                            pending.extend(zproj_thunks(qb_p - 1))
                        if hp_p == 0 and qb_p == QB - 1:
                            # last qb: heads 0-1 can normalize a phase early,
                            # shortening the tail's elementwise chain
                            finalize_half(QB - 1, 0)
                    prev = (qb, hp, pts, None)
                # drain the last pair
                qb_p, hp_p, pts_p, _ = prev
                pos_p = [ps_pv.tile([VW, QBS], F32, tag="po", name="po")
                         for _ in range(2)]
                for kc in range(KC):
                    for hh in range(2):
                        h = hp_p * 2 + hh
                        nc.tensor.matmul(
                            pos_p[hh][:], va[:, kc, h, :],
                            pts_p[:, kc * 2 * QBS + hh * QBS:
                                  kc * 2 * QBS + (hh + 1) * QBS],
                            start=(kc == 0), stop=(kc == KC - 1))
                    pump(2)
                pv_finalize(qb_p, hp_p, pts_p, pos_p)
                finalize_half(qb_p, 1)
                pump(len(pending))
                for th in zproj_thunks(qb_p):
                    th()

    _split_waits(nc)
    return nc


def _get_built():
    global _BUILT
    if _BUILT is None:
        _BUILT = _build()
    return _BUILT


def kernel(x, Wqkv, Wg, Wout, mask=None, **_ignored):
    """Full inputs in, full output out. mask is all-ones by construction and
    is ignored (attention over an all-true mask is mask-free)."""
    from concourse.bass_utils import run_bass_kernel_spmd

    import ml_dtypes
    bf16 = ml_dtypes.bfloat16
    x = np.asarray(x, dtype=np.float32).astype(bf16)
    Wqkv = np.asarray(Wqkv, dtype=np.float32).astype(bf16)
    Wg = np.asarray(Wg, dtype=np.float32).astype(bf16)
    Wout = np.asarray(Wout, dtype=np.float32).astype(bf16)
    b = x.shape[0]

    # Wg [dim, w] -> [128, (chunk, w)] partition-tiled for a fast plain DMA
    wg_host = np.ascontiguousarray(
        Wg.reshape(CB, P, W).transpose(1, 0, 2).reshape(P, CB * W))

    in_maps = []
    for c in range(8):
        beta, g = c // 4, c % 4
        cols = []
        for q in range(3):
            blk = Wqkv[:, (q * 16 + 4 * g) * 256:(q * 16 + 4 * g + 4) * 256]
            # local column packing (h, d, w) -> (w, h, d): stationary matmul
            # slices become contiguous 128-column runs
            blk = blk.reshape(DIM, NH_LOC, DH, W).transpose(0, 3, 1, 2)
            cols.append(blk.reshape(DIM, FEAT * W))
        wo = Wout[g * 256:(g + 1) * 256, :]
        # (eh, e, w) -> (eh, w, e): moving operand streams contiguously
        wo = wo.reshape(FEAT, 2, 512, W).transpose(0, 1, 3, 2)
        in_maps.append({
            "x": np.ascontiguousarray(x[beta]),
            "wqkv": np.ascontiguousarray(np.concatenate(cols, axis=1)),
            "wg": wg_host,
            "wout": np.ascontiguousarray(wo.reshape(FEAT, DIM * W)),
        })

    nc = _get_built()
    trace = bool(int(os.environ.get("KBENCH_TRACE", "0")))
    res = run_bass_kernel_spmd(nc, in_maps, core_ids=list(range(8)), trace=trace)
    kernel.last_exec_time_ns = res.exec_time_ns

    out = np.zeros((b, N_TOK, DIM), dtype=np.float32)
    for c in range(8):
        out[c // 4] += res.results[c]["out"].astype(np.float32)
    return out
